# revision 14
# baseline (speedup 1.0000x reference)
"""NEAT layer kernel for Trainium2 (8 NeuronCores, pure data parallel).

Math (per reference): vals starts as x [B,64]; for each layer li with
(src, w, b): z = sum_k vals[:, src[n,k]] * w[n,k] + b[n]; out = sigmoid(5*z);
vals = concat(vals, out). Output = layer-3 out [B,10].

v2 design (vs the 82us fp32r baseline): the baseline was bound by the
Activation engine (81 small ACTs ~673ns each ~ 55us) and by fp32r matmul
moving-operand streaming (~676ns per 512-col pass).

1. All matmul operands are fp16: moving streams at 1 col/cycle @2.4GHz
   (~213ns per 512 pass) vs fp32r's ~2x, and LDWEIGHTS gets FWL. fp16
   (not bf16): 10 mantissa bits keep the 4-layer pipeline at ~5e-3 rel
   err (bf16's 7 bits blow the 2e-2 budget at 4e-2, host-verified).
2. Biases are folded into the matmuls via a ones-row (row 64 of the x
   tile carries 1.0; stationary row 64 carries b (the ACT scale multiplies the whole psum by 5)). ACT bias APs go
   away, which makes sigmoids of DIFFERENT layers mergeable into one
   wide ACT instruction: per pipeline step one [128, 1024] ACT covers
   sigmoid of z0(t) and z1(t-1) sitting in adjacent PSUM banks of one
   [128,1024] tile. ACT cost is (FD+222)cyc/1.2GHz, so fewer+wider
   instructions ~halve ACT busy time.
3. Narrow layers are partition-packed: z23 = [z2(32); z3partial(10)] for
   TWO chunks lives in one [84, 512] PSUM tile (col-block stationaries
   with zero halves keep every matmul dst at base partition 0), one ACT
   per chunk pair. z3 output accumulates 8 chunks in one [80, 512] PSUM
   bank (col-block stationaries; identity rows fold the raw z3 partial,
   which a DVE copy stages next to sigma(z2)); 2 ACTs total for o3.
4. x arrives as one [65, 8192] fp16 tile (row 64 = ones) in 4 chunked
   DMAs on the SP HWDGE ring while the weight blob rides the ACT HWDGE
   ring concurrently; output DMAs share the SP FIFO (so the teardown
   Drain can wait a single lane that dominates everything).

Per 512-sample chunk the PE still runs 7 passes (z0x, z1x, z1o0, z23x,
z23o0, z23o1, q3fold) but at bf16 speed. Batch 65536 = 8 cores x 16
chunks of 512.
"""

import sys

sys.path.insert(0, "/opt/trn_rl_repo")

import numpy as np

import concourse.bass as bass
import concourse.mybir as mybir
from concourse.tile import TileContext

BATCH = 65536
IN_DIM = 64
FAN_IN = 16
GAIN = 5.0
N_CORES = 8
BC = BATCH // N_CORES          # 8192 samples per core
CHUNK = 512
N_CHUNKS = BC // CHUNK         # 16

# Node index blocks in the accumulated `vals` array.
X_LO, X_HI = 0, 64
H0_LO, H0_HI = 64, 192
H1_LO, H1_HI = 192, 288
H2_LO, H2_HI = 288, 320

F32 = mybir.dt.float32
F32R = mybir.dt.float32r

# float16: 10 mantissa bits (full-pipeline rel err ~5e-3 vs bf16's 4e-2,
# host-verified) at bf16-class PE speed (1 cyc/col moving stream + FWL).
DT = mybir.dt.float16
NP_DT = np.float16
DEBUG = False

# Weight blob column layout [128, WCOLS].
C_W0 = 0          # [65,128] x->z0, row 64 = 5*b0
C_W1X = 128       # [65,128] x->z1 (cols 0..95; 96..127 zero), row 64 = 5*b1
C_W1H = 256       # [128,128] o0->z1 (cols 0..95)
C_WCX = 384       # 2x [65,106]  x->z23 halves, bias in row 64
C_WCH0 = 596      # 2x [128,106] o0->z23 halves
C_WCH1 = 808      # 2x [96,106]  o1->z23 halves
C_WQ = 1020       # 8x [42,80] z23-half -> o3 col-blocks (identity fold rows)
WCOLS = C_WQ + 8 * 80  # 1660
# z23 pair tile rows (all AP base partitions must be in {0, 32, 64}):
#   half 0 (even chunk): rows 0..31 = z2, 32..41 = z3p, 42..63 = zero fill
#   half 1 (odd chunk):  rows 64..73 = z3p, 74..105 = z2
CROWS = 106


def _round_dt(a: np.ndarray) -> np.ndarray:
    return np.asarray(a, np.float32).astype(np.float16)


def _scatter(dst: np.ndarray, src: np.ndarray, w: np.ndarray, lo: int, hi: int,
             col_off: int) -> None:
    """dst[src[n,k]-lo, n+col_off] += w[n,k] for src entries in [lo,hi)."""
    n, k = src.shape
    cols = np.repeat(np.arange(n, dtype=np.int64), k) + col_off
    s = src.ravel().astype(np.int64)
    v = w.ravel().astype(np.float64)
    m = (s >= lo) & (s < hi)
    np.add.at(dst, (s[m] - lo, cols[m]), v[m])


def _build_wblob(inputs: dict) -> np.ndarray:
    wb = np.zeros([128, WCOLS], np.float64)

    # z0: x -> 128 nodes (bias rows carry b, NOT 5b: ACT scale=5 hits them too)
    W0 = np.zeros([65, 128], np.float64)
    _scatter(W0, inputs["src0"], inputs["w0"], X_LO, X_HI, 0)
    W0[64, :] = np.asarray(inputs["b0"], np.float64)
    wb[0:65, C_W0:C_W0 + 128] = W0

    # z1: x + o0 -> 96 nodes (cols 0..95 of a 128-wide dst)
    W1X = np.zeros([65, 128], np.float64)
    _scatter(W1X, inputs["src1"], inputs["w1"], X_LO, X_HI, 0)
    W1X[64, 0:96] = np.asarray(inputs["b1"], np.float64)
    wb[0:65, C_W1X:C_W1X + 128] = W1X
    W1H = np.zeros([128, 128], np.float64)
    _scatter(W1H, inputs["src1"], inputs["w1"], H0_LO, H0_HI, 0)
    wb[0:128, C_W1H:C_W1H + 128] = W1H

    # z23: x + o0 + o1 -> z2(32)+z3p(10) per chunk-pair half h.
    z2off = {0: 0, 1: 74}
    z3off = {0: 32, 1: 64}
    b2 = np.asarray(inputs["b2"], np.float64)
    b3 = np.asarray(inputs["b3"], np.float64)
    for h in (0, 1):
        WCX = np.zeros([65, CROWS], np.float64)
        _scatter(WCX, inputs["src2"], inputs["w2"], X_LO, X_HI, z2off[h])
        _scatter(WCX, inputs["src3"], inputs["w3"], X_LO, X_HI, z3off[h])
        WCX[64, z2off[h]:z2off[h] + 32] = b2
        WCX[64, z3off[h]:z3off[h] + 10] = b3
        wb[0:65, C_WCX + CROWS * h:C_WCX + CROWS * (h + 1)] = WCX

        WCH0 = np.zeros([128, CROWS], np.float64)
        _scatter(WCH0, inputs["src2"], inputs["w2"], H0_LO, H0_HI, z2off[h])
        _scatter(WCH0, inputs["src3"], inputs["w3"], H0_LO, H0_HI, z3off[h])
        wb[0:128, C_WCH0 + CROWS * h:C_WCH0 + CROWS * (h + 1)] = WCH0

        WCH1 = np.zeros([96, CROWS], np.float64)
        _scatter(WCH1, inputs["src2"], inputs["w2"], H1_LO, H1_HI, z2off[h])
        _scatter(WCH1, inputs["src3"], inputs["w3"], H1_LO, H1_HI, z3off[h])
        wb[0:96, C_WCH1 + CROWS * h:C_WCH1 + CROWS * (h + 1)] = WCH1

    # q3 fold: z23-half rows -> o3, one 10-col block per chunk slot k.
    # Moving rows for even chunks: [z2(32); z3p(10)]; odd: [z3p(10); z2(32)].
    WH23 = np.zeros([32, 10], np.float64)
    _scatter(WH23, inputs["src3"], inputs["w3"], H2_LO, H2_HI, 0)
    WQ = {0: np.zeros([42, 10], np.float64), 1: np.zeros([42, 10], np.float64)}
    WQ[0][0:32] = WH23
    WQ[0][32:42] = np.eye(10)
    WQ[1][0:10] = np.eye(10)
    WQ[1][10:42] = WH23
    # Stationary base partition must match the moving operand's (64 for
    # odd chunks, whose z23 half lives at rows 64..105).
    for k in range(8):
        base = C_WQ + 80 * k + 10 * k
        rb = 64 * (k % 2)
        wb[rb:rb + 42, base:base + 10] = WQ[k % 2]

    return np.ascontiguousarray(
        _round_dt(wb).astype(NP_DT))


def build_nc() -> bass.Bass:
    nc = bass.Bass()
    wd = nc.declare_dram_parameter("wblob", [128, WCOLS], DT, isOutput=False)
    xd = nc.declare_dram_parameter("xin", [65, BC], DT, isOutput=False)
    o3d = [nc.declare_dram_parameter(f"o3_{g}", [80, CHUNK], F32,
                                     isOutput=True) for g in range(2)]
    if DEBUG:
        dbg_ab = nc.declare_dram_parameter("dbg_ab", [128, 4 * CHUNK], DT,
                                           isOutput=True)
        dbg_c = nc.declare_dram_parameter("dbg_c", [CROWS, CHUNK], DT,
                                          isOutput=True)

    SIG = mybir.ActivationFunctionType.Sigmoid
    NX = BC // 4                # x DMA slice width (2048)

    def cs(c):
        return slice(c * CHUNK, (c + 1) * CHUNK)

    with TileContext(nc) as tc:
        with (
            tc.tile_pool(name="persist", bufs=1) as pp,
            tc.tile_pool(name="pab", bufs=2, space="PSUM") as pab,
            tc.tile_pool(name="pc", bufs=2, space="PSUM") as pc,
            tc.tile_pool(name="pq", bufs=2, space="PSUM") as pq,
        ):
            wb = pp.tile([128, WCOLS], DT)
            xt = pp.tile([65, BC], DT)
            st_ab = pp.tile([128, 17 * 2 * CHUNK], DT)   # per step: o0 | o1
            st_c = pp.tile([CROWS, 8 * CHUNK], DT)       # per pair: z23 sigmas
            o3sb = pp.tile([80, 2 * CHUNK], F32)

            # Weights on the ACT HWDGE ring; x slices on the SP ring.
            nc.scalar.dma_start(out=wb[:], in_=wd[:])
            for s in range(4):
                nc.sync.dma_start(out=xt[:, s * NX:(s + 1) * NX],
                                  in_=xd[:, s * NX:(s + 1) * NX])

            def o0ap(c, rows=128):
                return st_ab[0:rows, c * 1024:c * 1024 + CHUNK]

            def o1ap(c, rows=96):
                base = (c + 1) * 1024 + CHUNK
                return st_ab[0:rows, base:base + CHUNK]

            AB, C, Q = {}, {}, {}
            for t in range(N_CHUNKS + 6):
                # 1. merged sigmoid over AB_{t-1}: o0(t-1) | o1(t-2)
                if 1 <= t <= N_CHUNKS + 1:
                    lo = 0 if t - 1 <= N_CHUNKS - 1 else CHUNK
                    hi = 2 * CHUNK if 0 <= t - 2 else CHUNK
                    ab = AB[t - 1]
                    nc.scalar.activation(
                        st_ab[:, (t - 1) * 1024 + lo:(t - 1) * 1024 + hi],
                        ab[:, lo:hi], SIG, scale=GAIN)

                # 2. z0(t) and z1(t-1) matmuls into AB_t
                if t <= N_CHUNKS:
                    ab = AB[t] = pab.tile([128, 2 * CHUNK], F32, name="ab")
                    if t <= N_CHUNKS - 1:
                        nc.tensor.matmul(ab[:, 0:CHUNK],
                                         wb[0:65, C_W0:C_W0 + 128],
                                         xt[:, cs(t)], start=True, stop=True)
                    if 1 <= t:
                        c = t - 1
                        nc.tensor.matmul(ab[:, CHUNK:2 * CHUNK],
                                         wb[0:65, C_W1X:C_W1X + 128],
                                         xt[:, cs(c)], start=True, stop=False)
                        nc.tensor.matmul(ab[:, CHUNK:2 * CHUNK],
                                         wb[0:128, C_W1H:C_W1H + 128],
                                         o0ap(c), start=False, stop=True)

                # 3. q3 folds for the pair completed last step (p = (t-4)//2)
                if t >= 4 and (t - 4) % 2 == 0 and (p := (t - 4) // 2) <= 7:
                    for c in (2 * p, 2 * p + 1):
                        g, k = c // 8, c % 8
                        if k == 0:
                            Q[g] = pq.tile([80, CHUNK], F32, name="q")
                        h = c % 2
                        nc.tensor.matmul(
                            Q[g][:],
                            wb[64 * h:64 * h + 42,
                               C_WQ + 80 * k:C_WQ + 80 * (k + 1)],
                            st_c[64 * h:64 * h + 42,
                                 p * CHUNK:(p + 1) * CHUNK],
                            start=(k == 0), stop=(k == 7))
                        if k == 7:
                            nc.scalar.activation(
                                o3sb[:, g * CHUNK:(g + 1) * CHUNK],
                                Q[g][:], SIG, scale=GAIN)
                            nc.sync.dma_start(
                                out=o3d[g][:],
                                in_=o3sb[:, g * CHUNK:(g + 1) * CHUNK])

                # 4. z23(t-2) matmuls into C_p
                if 2 <= t <= N_CHUNKS + 1:
                    c = t - 2
                    p, h = c // 2, c % 2
                    if h == 0:
                        C[p] = pc.tile([CROWS, CHUNK], F32, name="c23")
                    cp = C[p]
                    nc.tensor.matmul(
                        cp[:], wb[0:65, C_WCX + CROWS * h:C_WCX + CROWS * (h + 1)],
                        xt[:, cs(c)], start=(h == 0), stop=False)
                    nc.tensor.matmul(
                        cp[:], wb[0:128, C_WCH0 + CROWS * h:C_WCH0 + CROWS * (h + 1)],
                        o0ap(c), start=False, stop=False)
                    nc.tensor.matmul(
                        cp[:], wb[0:96, C_WCH1 + CROWS * h:C_WCH1 + CROWS * (h + 1)],
                        o1ap(c), start=False, stop=(h == 1))
                    # 5. pair sigmoid + raw z3p staging copies
                    if h == 1:
                        nc.scalar.activation(
                            st_c[:, p * CHUNK:(p + 1) * CHUNK],
                            cp[:], SIG, scale=GAIN)
                        for rb in (32, 64):
                            nc.vector.tensor_copy(
                                st_c[rb:rb + 10, p * CHUNK:(p + 1) * CHUNK],
                                cp[rb:rb + 10, :])
                        if DEBUG and p == 0:
                            nc.gpsimd.dma_start(out=dbg_c[:],
                                                in_=st_c[:, 0:CHUNK])
                            nc.gpsimd.dma_start(out=dbg_ab[:],
                                                in_=st_ab[:, 2 * CHUNK:6 * CHUNK])
    _fix_sync_waits(nc)
    return nc


def _fix_sync_waits(nc: bass.Bass) -> None:
    """Walrus codegen has ONE sync-wait slot per MM/ACT/DRAIN instruction.
    The tile scheduler emits two waits in three places; each extra wait is
    provably redundant, so drop it:

    (a) A matmul WAW-waits its own PE lane (same PSUM bank, two buffers
    ago) next to a DMA wait. The PE is an in-order FIFO whose sem
    increments follow completion order, and a matmul's first PSUM write
    lands after the previous matmul's last one - program order already
    enforces the WAW.
    (b) A fold matmul waits both the pair-sigmoid ACT and the z3p DVE
    copies. The copies themselves wait that same ACT (WAW on st_c), so
    the DVE wait subsumes the ACT wait.
    (c) The teardown Drain waits every engine's final sem; prune to the
    last output DMA's lane: it waits the last group ACT (<- last fold <-
    everything upstream) and shares the SP HWDGE FIFO with every other
    DMA, so its completion dominates all lanes.
    """
    ins = list(nc.all_instructions())
    act_wait_of_dve = {}   # nth DVE update -> Activation wait value it held
    ndve = 0
    for i in ins:
        si = getattr(i, "sync_info", None)
        if si is None:
            continue
        if any(u.ant_name.startswith("DVE") for u in si.on_update):
            ndve += 1
            for w in si.on_wait:
                if w.ant_name.startswith("Activation"):
                    act_wait_of_dve[ndve] = w.wait_value
    for i in ins:
        tn = type(i).__name__
        si = i.sync_info
        if tn == "InstMatmult" and si and len(si.on_wait) > 1:
            pe = [w for w in si.on_wait if w.ant_name.startswith("PE")]
            act = [w for w in si.on_wait if w.ant_name.startswith("Activation")]
            dve = [w for w in si.on_wait if w.ant_name.startswith("DVE")]
            if pe and len(si.on_wait) - len(pe) == 1:
                si.on_wait = [w for w in si.on_wait if w not in pe]
            elif act and dve and len(si.on_wait) == 2:
                # check the DVE target's own Activation wait subsumes ours
                dtgt = dve[0].wait_value
                cover = max((v for n, v in act_wait_of_dve.items()
                             if n <= dtgt), default=-1)
                assert cover >= act[0].wait_value, (dtgt, cover, act[0])
                si.on_wait = dve
            assert len(si.on_wait) == 1, [w.ant_name for w in si.on_wait]
            i.sync_info = si
        elif tn == "InstDrain" and si and len(si.on_wait) > 1:
            last_upd = None
            for j in ins:
                if type(j).__name__ == "InstDMACopy" and j.sync_info and \
                        j.outs and "o3_" in str(j.outs[0]):
                    for u in j.sync_info.on_update:
                        last_upd = u.ant_name
            si.on_wait = [w for w in si.on_wait if w.ant_name == last_upd]
            assert len(si.on_wait) == 1, si.on_wait
            i.sync_info = si


def make_in_maps(inputs: dict) -> list[dict]:
    wblob = _build_wblob(inputs)
    x = np.asarray(inputs["x"], np.float32)
    in_maps = []
    for i in range(N_CORES):
        xt = np.empty((65, BC), np.float32)
        xt[0:64, :] = x[i * BC:(i + 1) * BC, :].T
        xt[64, :] = 1.0
        in_maps.append({"wblob": wblob,
                        "xin": np.ascontiguousarray(_round_dt(xt).astype(NP_DT))})
    return in_maps


def assemble_output(results: list[dict]) -> np.ndarray:
    y = np.empty((BATCH, 10), np.float32)
    for i in range(N_CORES):
        for g in range(2):
            o = np.asarray(results[i][f"o3_{g}"], np.float32)  # [80, 512]
            # row 10k+j, col s  ->  sample (8g+k)*512+s, node j
            o = o.reshape(8, 10, CHUNK)
            for k in range(8):
                lo = i * BC + (8 * g + k) * CHUNK
                y[lo:lo + CHUNK, :] = o[k].T
    return y


def assemble_output_core0(results: list[dict]) -> np.ndarray:
    y = np.empty((BC, 10), np.float32)
    for g in range(2):
        o = np.asarray(results[0][f"o3_{g}"], np.float32).reshape(8, 10, CHUNK)
        for k in range(8):
            lo = (8 * g + k) * CHUNK
            y[lo:lo + CHUNK, :] = o[k].T
    return y


def kernel(**inputs: np.ndarray) -> np.ndarray:
    from concourse.bass_utils import run_bass_kernel_spmd

    nc = build_nc()
    in_maps = make_in_maps(inputs)
    res = run_bass_kernel_spmd(nc, in_maps, list(range(N_CORES)))
    return assemble_output(res.results)


# revision 15
# speedup vs baseline: 1.1467x; 1.1467x over previous
"""NEAT layer kernel for Trainium2 (8 NeuronCores, pure data parallel).

Math (per reference): vals starts as x [B,64]; for each layer li with
(src, w, b): z = sum_k vals[:, src[n,k]] * w[n,k] + b[n]; out = sigmoid(5*z);
vals = concat(vals, out). Output = layer-3 out [B,10].

v2 design (vs the 82us fp32r baseline): the baseline was bound by the
Activation engine (81 small ACTs ~673ns each ~ 55us) and by fp32r matmul
moving-operand streaming (~676ns per 512-col pass).

1. All matmul operands are fp16: moving streams at 1 col/cycle @2.4GHz
   (~213ns per 512 pass) vs fp32r's ~2x, and LDWEIGHTS gets FWL. fp16
   (not bf16): 10 mantissa bits keep the 4-layer pipeline at ~5e-3 rel
   err (bf16's 7 bits blow the 2e-2 budget at 4e-2, host-verified).
2. Biases are folded into the matmuls via a ones-row (row 64 of the x
   tile carries 1.0; stationary row 64 carries b (the ACT scale multiplies the whole psum by 5)). ACT bias APs go
   away, which makes sigmoids of DIFFERENT layers mergeable into one
   wide ACT instruction: per pipeline step one [128, 1024] ACT covers
   sigmoid of z0(t) and z1(t-1) sitting in adjacent PSUM banks of one
   [128,1024] tile. ACT cost is (FD+222)cyc/1.2GHz, so fewer+wider
   instructions ~halve ACT busy time.
3. Narrow layers are partition-packed: z23 = [z2(32); z3partial(10)] for
   TWO chunks lives in one [84, 512] PSUM tile (col-block stationaries
   with zero halves keep every matmul dst at base partition 0), one ACT
   per chunk pair. z3 output accumulates 8 chunks in one [80, 512] PSUM
   bank (col-block stationaries; identity rows fold the raw z3 partial,
   which a DVE copy stages next to sigma(z2)); 2 ACTs total for o3.
4. x arrives as one [65, 8192] fp16 tile (row 64 = ones) in 4 chunked
   DMAs on the SP HWDGE ring while the weight blob rides the ACT HWDGE
   ring concurrently; output DMAs share the SP FIFO (so the teardown
   Drain can wait a single lane that dominates everything).

Per 512-sample chunk the PE still runs 7 passes (z0x, z1x, z1o0, z23x,
z23o0, z23o1, q3fold) but at bf16 speed. Batch 65536 = 8 cores x 16
chunks of 512.
"""

import sys

sys.path.insert(0, "/opt/trn_rl_repo")

import numpy as np

import concourse.bass as bass
import concourse.mybir as mybir
from concourse.tile import TileContext

BATCH = 65536
IN_DIM = 64
FAN_IN = 16
GAIN = 5.0
N_CORES = 8
BC = BATCH // N_CORES          # 8192 samples per core
CHUNK = 512
N_CHUNKS = BC // CHUNK         # 16

# Node index blocks in the accumulated `vals` array.
X_LO, X_HI = 0, 64
H0_LO, H0_HI = 64, 192
H1_LO, H1_HI = 192, 288
H2_LO, H2_HI = 288, 320

F32 = mybir.dt.float32
F32R = mybir.dt.float32r

# float16: 10 mantissa bits (full-pipeline rel err ~5e-3 vs bf16's 4e-2,
# host-verified) at bf16-class PE speed (1 cyc/col moving stream + FWL).
DT = mybir.dt.float16
NP_DT = np.float16
DEBUG = False
MERGED_ACT = True

# Weight blob column layout [128, WCOLS].
C_W0 = 0          # [65,128] x->z0, row 64 = 5*b0
C_W1X = 128       # [65,128] x->z1 (cols 0..95; 96..127 zero), row 64 = 5*b1
C_W1H = 256       # [128,128] o0->z1 (cols 0..95)
C_WCX = 384       # 2x [65,106]  x->z23 halves, bias in row 64
C_WCH0 = 596      # 2x [128,106] o0->z23 halves
C_WCH1 = 808      # 2x [96,106]  o1->z23 halves
C_WQ = 1020       # 8x [42,80] z23-half -> o3 col-blocks (identity fold rows)
WCOLS = C_WQ + 8 * 80  # 1660
# z23 pair tile rows (all AP base partitions must be in {0, 32, 64}):
#   half 0 (even chunk): rows 0..31 = z2, 32..41 = z3p, 42..63 = zero fill
#   half 1 (odd chunk):  rows 64..73 = z3p, 74..105 = z2
CROWS = 106


def _round_dt(a: np.ndarray) -> np.ndarray:
    return np.asarray(a, np.float32).astype(np.float16)


def _scatter(dst: np.ndarray, src: np.ndarray, w: np.ndarray, lo: int, hi: int,
             col_off: int) -> None:
    """dst[src[n,k]-lo, n+col_off] += w[n,k] for src entries in [lo,hi)."""
    n, k = src.shape
    cols = np.repeat(np.arange(n, dtype=np.int64), k) + col_off
    s = src.ravel().astype(np.int64)
    v = w.ravel().astype(np.float64)
    m = (s >= lo) & (s < hi)
    np.add.at(dst, (s[m] - lo, cols[m]), v[m])


def _build_wblob(inputs: dict) -> np.ndarray:
    wb = np.zeros([128, WCOLS], np.float64)

    # z0: x -> 128 nodes (bias rows carry b, NOT 5b: ACT scale=5 hits them too)
    W0 = np.zeros([65, 128], np.float64)
    _scatter(W0, inputs["src0"], inputs["w0"], X_LO, X_HI, 0)
    W0[64, :] = np.asarray(inputs["b0"], np.float64)
    wb[0:65, C_W0:C_W0 + 128] = W0

    # z1: x + o0 -> 96 nodes (cols 0..95 of a 128-wide dst)
    W1X = np.zeros([65, 128], np.float64)
    _scatter(W1X, inputs["src1"], inputs["w1"], X_LO, X_HI, 0)
    W1X[64, 0:96] = np.asarray(inputs["b1"], np.float64)
    wb[0:65, C_W1X:C_W1X + 128] = W1X
    W1H = np.zeros([128, 128], np.float64)
    _scatter(W1H, inputs["src1"], inputs["w1"], H0_LO, H0_HI, 0)
    wb[0:128, C_W1H:C_W1H + 128] = W1H

    # z23: x + o0 + o1 -> z2(32)+z3p(10) per chunk-pair half h.
    z2off = {0: 0, 1: 74}
    z3off = {0: 32, 1: 64}
    b2 = np.asarray(inputs["b2"], np.float64)
    b3 = np.asarray(inputs["b3"], np.float64)
    for h in (0, 1):
        WCX = np.zeros([65, CROWS], np.float64)
        _scatter(WCX, inputs["src2"], inputs["w2"], X_LO, X_HI, z2off[h])
        _scatter(WCX, inputs["src3"], inputs["w3"], X_LO, X_HI, z3off[h])
        WCX[64, z2off[h]:z2off[h] + 32] = b2
        WCX[64, z3off[h]:z3off[h] + 10] = b3
        wb[0:65, C_WCX + CROWS * h:C_WCX + CROWS * (h + 1)] = WCX

        WCH0 = np.zeros([128, CROWS], np.float64)
        _scatter(WCH0, inputs["src2"], inputs["w2"], H0_LO, H0_HI, z2off[h])
        _scatter(WCH0, inputs["src3"], inputs["w3"], H0_LO, H0_HI, z3off[h])
        wb[0:128, C_WCH0 + CROWS * h:C_WCH0 + CROWS * (h + 1)] = WCH0

        WCH1 = np.zeros([96, CROWS], np.float64)
        _scatter(WCH1, inputs["src2"], inputs["w2"], H1_LO, H1_HI, z2off[h])
        _scatter(WCH1, inputs["src3"], inputs["w3"], H1_LO, H1_HI, z3off[h])
        wb[0:96, C_WCH1 + CROWS * h:C_WCH1 + CROWS * (h + 1)] = WCH1

    # q3 fold: z23-half rows -> o3, one 10-col block per chunk slot k.
    # Moving rows for even chunks: [z2(32); z3p(10)]; odd: [z3p(10); z2(32)].
    WH23 = np.zeros([32, 10], np.float64)
    _scatter(WH23, inputs["src3"], inputs["w3"], H2_LO, H2_HI, 0)
    WQ = {0: np.zeros([42, 10], np.float64), 1: np.zeros([42, 10], np.float64)}
    WQ[0][0:32] = WH23
    WQ[0][32:42] = np.eye(10)
    WQ[1][0:10] = np.eye(10)
    WQ[1][10:42] = WH23
    # Stationary base partition must match the moving operand's (64 for
    # odd chunks, whose z23 half lives at rows 64..105).
    for k in range(8):
        base = C_WQ + 80 * k + 10 * k
        rb = 64 * (k % 2)
        wb[rb:rb + 42, base:base + 10] = WQ[k % 2]

    return np.ascontiguousarray(
        _round_dt(wb).astype(NP_DT))


def build_nc() -> bass.Bass:
    nc = bass.Bass()
    wd = nc.declare_dram_parameter("wblob", [128, WCOLS], DT, isOutput=False)
    xd = nc.declare_dram_parameter("xin", [65, BC], DT, isOutput=False)
    o3d = [nc.declare_dram_parameter(f"o3_{g}", [80, CHUNK], F32,
                                     isOutput=True) for g in range(2)]
    if DEBUG:
        dbg_ab = nc.declare_dram_parameter("dbg_ab", [128, 4 * CHUNK], DT,
                                           isOutput=True)
        dbg_c = nc.declare_dram_parameter("dbg_c", [CROWS, CHUNK], DT,
                                          isOutput=True)

    SIG = mybir.ActivationFunctionType.Sigmoid
    NX = BC // 4                # x DMA slice width (2048)

    def cs(c):
        return slice(c * CHUNK, (c + 1) * CHUNK)

    with TileContext(nc) as tc:
        with (
            tc.tile_pool(name="persist", bufs=1) as pp,
            tc.tile_pool(name="pab", bufs=2, space="PSUM") as pab,
            tc.tile_pool(name="pc", bufs=2, space="PSUM") as pc,
            tc.tile_pool(name="pq", bufs=2, space="PSUM") as pq,
        ):
            wb = pp.tile([128, WCOLS], DT)
            xt = pp.tile([65, BC], DT)
            st_ab = pp.tile([128, 17 * 2 * CHUNK], DT)   # per step: o0 | o1
            st_c = pp.tile([CROWS, 8 * CHUNK], DT)       # per pair: z23 sigmas
            o3sb = pp.tile([80, 2 * CHUNK], F32)

            # Weights on the ACT HWDGE ring; x slices on the SP ring.
            nc.scalar.dma_start(out=wb[:], in_=wd[:])
            for s in range(4):
                nc.sync.dma_start(out=xt[:, s * NX:(s + 1) * NX],
                                  in_=xd[:, s * NX:(s + 1) * NX])

            def o0ap(c, rows=128):
                return st_ab[0:rows, c * 1024:c * 1024 + CHUNK]

            def o1ap(c, rows=96):
                base = (c + 1) * 1024 + CHUNK
                return st_ab[0:rows, base:base + CHUNK]

            AB, C, Q = {}, {}, {}
            for t in range(N_CHUNKS + 6):
                # 1. merged sigmoid over AB_{t-1}: o0(t-1) | o1(t-2)
                if 1 <= t <= N_CHUNKS + 1:
                    lo = 0 if t - 1 <= N_CHUNKS - 1 else CHUNK
                    hi = 2 * CHUNK if 0 <= t - 2 else CHUNK
                    ab = AB[t - 1]
                    if MERGED_ACT:
                        nc.scalar.activation(
                            st_ab[:, (t - 1) * 1024 + lo:(t - 1) * 1024 + hi],
                            ab[:, lo:hi], SIG, scale=GAIN)
                    else:
                        for s0 in range(lo, hi, CHUNK):
                            nc.scalar.activation(
                                st_ab[:, (t - 1) * 1024 + s0:
                                      (t - 1) * 1024 + s0 + CHUNK],
                                ab[:, s0:s0 + CHUNK], SIG, scale=GAIN)

                # 2. z0(t) and z1(t-1) matmuls into AB_t
                if t <= N_CHUNKS:
                    ab = AB[t] = pab.tile([128, 2 * CHUNK], F32, name="ab")
                    if t <= N_CHUNKS - 1:
                        nc.tensor.matmul(ab[:, 0:CHUNK],
                                         wb[0:65, C_W0:C_W0 + 128],
                                         xt[:, cs(t)], start=True, stop=True)
                    if 1 <= t:
                        c = t - 1
                        nc.tensor.matmul(ab[:, CHUNK:2 * CHUNK],
                                         wb[0:65, C_W1X:C_W1X + 128],
                                         xt[:, cs(c)], start=True, stop=False)
                        nc.tensor.matmul(ab[:, CHUNK:2 * CHUNK],
                                         wb[0:128, C_W1H:C_W1H + 128],
                                         o0ap(c), start=False, stop=True)

                # 3. q3 folds for the pair completed last step (p = (t-4)//2)
                if t >= 4 and (t - 4) % 2 == 0 and (p := (t - 4) // 2) <= 7:
                    for c in (2 * p, 2 * p + 1):
                        g, k = c // 8, c % 8
                        if k == 0:
                            Q[g] = pq.tile([80, CHUNK], F32, name="q")
                        h = c % 2
                        nc.tensor.matmul(
                            Q[g][:],
                            wb[64 * h:64 * h + 42,
                               C_WQ + 80 * k:C_WQ + 80 * (k + 1)],
                            st_c[64 * h:64 * h + 42,
                                 p * CHUNK:(p + 1) * CHUNK],
                            start=(k == 0), stop=(k == 7))
                        if k == 7:
                            nc.scalar.activation(
                                o3sb[:, g * CHUNK:(g + 1) * CHUNK],
                                Q[g][:], SIG, scale=GAIN)
                            nc.sync.dma_start(
                                out=o3d[g][:],
                                in_=o3sb[:, g * CHUNK:(g + 1) * CHUNK])

                # 4. z23(t-2) matmuls into C_p
                if 2 <= t <= N_CHUNKS + 1:
                    c = t - 2
                    p, h = c // 2, c % 2
                    if h == 0:
                        C[p] = pc.tile([CROWS, CHUNK], F32, name="c23")
                    cp = C[p]
                    nc.tensor.matmul(
                        cp[:], wb[0:65, C_WCX + CROWS * h:C_WCX + CROWS * (h + 1)],
                        xt[:, cs(c)], start=(h == 0), stop=False)
                    nc.tensor.matmul(
                        cp[:], wb[0:128, C_WCH0 + CROWS * h:C_WCH0 + CROWS * (h + 1)],
                        o0ap(c), start=False, stop=False)
                    nc.tensor.matmul(
                        cp[:], wb[0:96, C_WCH1 + CROWS * h:C_WCH1 + CROWS * (h + 1)],
                        o1ap(c), start=False, stop=(h == 1))
                    # 5. pair sigmoid + raw z3p staging copies
                    if h == 1:
                        nc.scalar.activation(
                            st_c[:, p * CHUNK:(p + 1) * CHUNK],
                            cp[:], SIG, scale=GAIN)
                        for rb in (32, 64):
                            nc.vector.tensor_copy(
                                st_c[rb:rb + 10, p * CHUNK:(p + 1) * CHUNK],
                                cp[rb:rb + 10, :])
                        if DEBUG and p == 0:
                            nc.gpsimd.dma_start(out=dbg_c[:],
                                                in_=st_c[:, 0:CHUNK])
                            nc.gpsimd.dma_start(out=dbg_ab[:],
                                                in_=st_ab[:, 2 * CHUNK:6 * CHUNK])
    _fix_sync_waits(nc)
    return nc


def _fix_sync_waits(nc: bass.Bass) -> None:
    """Walrus codegen has ONE sync-wait slot per MM/ACT/DRAIN instruction.
    The tile scheduler emits two waits in three places; each extra wait is
    provably redundant, so drop it:

    (a) A matmul WAW-waits its own PE lane (same PSUM bank, two buffers
    ago) next to a DMA wait. The PE is an in-order FIFO whose sem
    increments follow completion order, and a matmul's first PSUM write
    lands after the previous matmul's last one - program order already
    enforces the WAW.
    (b) A fold matmul waits both the pair-sigmoid ACT and the z3p DVE
    copies. The copies themselves wait that same ACT (WAW on st_c), so
    the DVE wait subsumes the ACT wait.
    (c) The teardown Drain waits every engine's final sem; prune to the
    last output DMA's lane: it waits the last group ACT (<- last fold <-
    everything upstream) and shares the SP HWDGE FIFO with every other
    DMA, so its completion dominates all lanes.
    """
    ins = list(nc.all_instructions())
    act_wait_of_dve = {}   # nth DVE update -> Activation wait value it held
    ndve = 0
    for i in ins:
        si = getattr(i, "sync_info", None)
        if si is None:
            continue
        if any(u.ant_name.startswith("DVE") for u in si.on_update):
            ndve += 1
            for w in si.on_wait:
                if w.ant_name.startswith("Activation"):
                    act_wait_of_dve[ndve] = w.wait_value
    for i in ins:
        tn = type(i).__name__
        si = i.sync_info
        if tn == "InstMatmult" and si and len(si.on_wait) > 1:
            pe = [w for w in si.on_wait if w.ant_name.startswith("PE")]
            act = [w for w in si.on_wait if w.ant_name.startswith("Activation")]
            dve = [w for w in si.on_wait if w.ant_name.startswith("DVE")]
            if pe and len(si.on_wait) - len(pe) == 1:
                si.on_wait = [w for w in si.on_wait if w not in pe]
            elif act and dve and len(si.on_wait) == 2:
                # check the DVE target's own Activation wait subsumes ours
                dtgt = dve[0].wait_value
                cover = max((v for n, v in act_wait_of_dve.items()
                             if n <= dtgt), default=-1)
                assert cover >= act[0].wait_value, (dtgt, cover, act[0])
                si.on_wait = dve
            assert len(si.on_wait) == 1, [w.ant_name for w in si.on_wait]
            i.sync_info = si
        elif tn == "InstDrain" and si and len(si.on_wait) > 1:
            last_upd = None
            for j in ins:
                if type(j).__name__ == "InstDMACopy" and j.sync_info and \
                        j.outs and "o3_" in str(j.outs[0]):
                    for u in j.sync_info.on_update:
                        last_upd = u.ant_name
            si.on_wait = [w for w in si.on_wait if w.ant_name == last_upd]
            assert len(si.on_wait) == 1, si.on_wait
            i.sync_info = si


def make_in_maps(inputs: dict) -> list[dict]:
    wblob = _build_wblob(inputs)
    x = np.asarray(inputs["x"], np.float32)
    in_maps = []
    for i in range(N_CORES):
        xt = np.empty((65, BC), np.float32)
        xt[0:64, :] = x[i * BC:(i + 1) * BC, :].T
        xt[64, :] = 1.0
        in_maps.append({"wblob": wblob,
                        "xin": np.ascontiguousarray(_round_dt(xt).astype(NP_DT))})
    return in_maps


def assemble_output(results: list[dict]) -> np.ndarray:
    y = np.empty((BATCH, 10), np.float32)
    for i in range(N_CORES):
        for g in range(2):
            o = np.asarray(results[i][f"o3_{g}"], np.float32)  # [80, 512]
            # row 10k+j, col s  ->  sample (8g+k)*512+s, node j
            o = o.reshape(8, 10, CHUNK)
            for k in range(8):
                lo = i * BC + (8 * g + k) * CHUNK
                y[lo:lo + CHUNK, :] = o[k].T
    return y


def assemble_output_core0(results: list[dict]) -> np.ndarray:
    y = np.empty((BC, 10), np.float32)
    for g in range(2):
        o = np.asarray(results[0][f"o3_{g}"], np.float32).reshape(8, 10, CHUNK)
        for k in range(8):
            lo = (8 * g + k) * CHUNK
            y[lo:lo + CHUNK, :] = o[k].T
    return y


def kernel(**inputs: np.ndarray) -> np.ndarray:
    from concourse.bass_utils import run_bass_kernel_spmd

    nc = build_nc()
    in_maps = make_in_maps(inputs)
    res = run_bass_kernel_spmd(nc, in_maps, list(range(N_CORES)))
    return assemble_output(res.results)


# revision 20
# speedup vs baseline: 1.1784x; 1.0276x over previous
"""NEAT layer kernel for Trainium2 (8 NeuronCores, pure data parallel).

Math (per reference): vals starts as x [B,64]; for each layer li with
(src, w, b): z = sum_k vals[:, src[n,k]] * w[n,k] + b[n]; out = sigmoid(5*z);
vals = concat(vals, out). Output = layer-3 out [B,10].

v2 design (vs the 82us fp32r baseline): the baseline was bound by the
Activation engine (81 small ACTs ~673ns each ~ 55us) and by fp32r matmul
moving-operand streaming (~676ns per 512-col pass).

1. All matmul operands are fp16: moving streams at 1 col/cycle @2.4GHz
   (~213ns per 512 pass) vs fp32r's ~2x, and LDWEIGHTS gets FWL. fp16
   (not bf16): 10 mantissa bits keep the 4-layer pipeline at ~5e-3 rel
   err (bf16's 7 bits blow the 2e-2 budget at 4e-2, host-verified).
2. Biases are folded into the matmuls via a ones-row (row 64 of the x
   tile carries 1.0; stationary row 64 carries b (the ACT scale multiplies the whole psum by 5)). ACT bias APs go
   away, which makes sigmoids of DIFFERENT layers mergeable into one
   wide ACT instruction: per pipeline step one [128, 1024] ACT covers
   sigmoid of z0(t) and z1(t-1) sitting in adjacent PSUM banks of one
   [128,1024] tile. ACT cost is (FD+222)cyc/1.2GHz, so fewer+wider
   instructions ~halve ACT busy time.
3. Narrow layers are partition-packed: z23 = [z2(32); z3partial(10)] for
   TWO chunks lives in one [84, 512] PSUM tile (col-block stationaries
   with zero halves keep every matmul dst at base partition 0), one ACT
   per chunk pair. z3 output accumulates 8 chunks in one [80, 512] PSUM
   bank (col-block stationaries; identity rows fold the raw z3 partial,
   which a DVE copy stages next to sigma(z2)); 2 ACTs total for o3.
4. x arrives as one [65, 8192] fp16 tile (row 64 = ones) in 4 chunked
   DMAs on the SP HWDGE ring while the weight blob rides the ACT HWDGE
   ring concurrently; output DMAs share the SP FIFO (so the teardown
   Drain can wait a single lane that dominates everything).

Per 512-sample chunk the PE still runs 7 passes (z0x, z1x, z1o0, z23x,
z23o0, z23o1, q3fold) but at bf16 speed. Batch 65536 = 8 cores x 16
chunks of 512.
"""

import sys

sys.path.insert(0, "/opt/trn_rl_repo")

import numpy as np

import concourse.bass as bass
import concourse.mybir as mybir
from concourse.tile import TileContext

BATCH = 65536
IN_DIM = 64
FAN_IN = 16
GAIN = 5.0
N_CORES = 8
BC = BATCH // N_CORES          # 8192 samples per core
CHUNK = 512
N_CHUNKS = BC // CHUNK         # 16

# Node index blocks in the accumulated `vals` array.
X_LO, X_HI = 0, 64
H0_LO, H0_HI = 64, 192
H1_LO, H1_HI = 192, 288
H2_LO, H2_HI = 288, 320

F32 = mybir.dt.float32
F32R = mybir.dt.float32r

# float16: 10 mantissa bits (full-pipeline rel err ~5e-3 vs bf16's 4e-2,
# host-verified) at bf16-class PE speed (1 cyc/col moving stream + FWL).
DT = mybir.dt.float16
NP_DT = np.float16
DEBUG = False
MERGED_ACT = True

# Weight blob column layout [128, WCOLS].
C_W0 = 0          # [65,128] x->z0, row 64 = 5*b0
C_W1X = 128       # [65,128] x->z1 (cols 0..95; 96..127 zero), row 64 = 5*b1
C_W1H = 256       # [128,128] o0->z1 (cols 0..95)
C_WCX = 384       # 2x [65,106]  x->z23 halves, bias in row 64
C_WCH0 = 596      # 2x [128,106] o0->z23 halves
C_WCH1 = 808      # 2x [96,106]  o1->z23 halves
C_WQ = 1020       # 8x [42,80] z23-half -> o3 col-blocks (identity fold rows)
WCOLS = C_WQ + 8 * 80  # 1660
# z23 pair tile rows (all AP base partitions must be in {0, 32, 64}):
#   half 0 (even chunk): rows 0..31 = z2, 32..41 = z3p, 42..63 = zero fill
#   half 1 (odd chunk):  rows 64..73 = z3p, 74..105 = z2
CROWS = 106


def _round_dt(a: np.ndarray) -> np.ndarray:
    return np.asarray(a, np.float32).astype(np.float16)


def _scatter(dst: np.ndarray, src: np.ndarray, w: np.ndarray, lo: int, hi: int,
             col_off: int) -> None:
    """dst[src[n,k]-lo, n+col_off] += w[n,k] for src entries in [lo,hi)."""
    n, k = src.shape
    cols = np.repeat(np.arange(n, dtype=np.int64), k) + col_off
    s = src.ravel().astype(np.int64)
    v = w.ravel().astype(np.float64)
    m = (s >= lo) & (s < hi)
    np.add.at(dst, (s[m] - lo, cols[m]), v[m])


def _build_wblob(inputs: dict) -> np.ndarray:
    wb = np.zeros([128, WCOLS], np.float64)

    # z0: x -> 128 nodes (bias rows carry b, NOT 5b: ACT scale=5 hits them too)
    W0 = np.zeros([65, 128], np.float64)
    _scatter(W0, inputs["src0"], inputs["w0"], X_LO, X_HI, 0)
    W0[64, :] = np.asarray(inputs["b0"], np.float64)
    wb[0:65, C_W0:C_W0 + 128] = W0

    # z1: x + o0 -> 96 nodes (cols 0..95 of a 128-wide dst)
    W1X = np.zeros([65, 128], np.float64)
    _scatter(W1X, inputs["src1"], inputs["w1"], X_LO, X_HI, 0)
    W1X[64, 0:96] = np.asarray(inputs["b1"], np.float64)
    wb[0:65, C_W1X:C_W1X + 128] = W1X
    W1H = np.zeros([128, 128], np.float64)
    _scatter(W1H, inputs["src1"], inputs["w1"], H0_LO, H0_HI, 0)
    wb[0:128, C_W1H:C_W1H + 128] = W1H

    # z23: x + o0 + o1 -> z2(32)+z3p(10) per chunk-pair half h.
    z2off = {0: 0, 1: 74}
    z3off = {0: 32, 1: 64}
    b2 = np.asarray(inputs["b2"], np.float64)
    b3 = np.asarray(inputs["b3"], np.float64)
    for h in (0, 1):
        WCX = np.zeros([65, CROWS], np.float64)
        _scatter(WCX, inputs["src2"], inputs["w2"], X_LO, X_HI, z2off[h])
        _scatter(WCX, inputs["src3"], inputs["w3"], X_LO, X_HI, z3off[h])
        WCX[64, z2off[h]:z2off[h] + 32] = b2
        WCX[64, z3off[h]:z3off[h] + 10] = b3
        wb[0:65, C_WCX + CROWS * h:C_WCX + CROWS * (h + 1)] = WCX

        WCH0 = np.zeros([128, CROWS], np.float64)
        _scatter(WCH0, inputs["src2"], inputs["w2"], H0_LO, H0_HI, z2off[h])
        _scatter(WCH0, inputs["src3"], inputs["w3"], H0_LO, H0_HI, z3off[h])
        wb[0:128, C_WCH0 + CROWS * h:C_WCH0 + CROWS * (h + 1)] = WCH0

        WCH1 = np.zeros([96, CROWS], np.float64)
        _scatter(WCH1, inputs["src2"], inputs["w2"], H1_LO, H1_HI, z2off[h])
        _scatter(WCH1, inputs["src3"], inputs["w3"], H1_LO, H1_HI, z3off[h])
        wb[0:96, C_WCH1 + CROWS * h:C_WCH1 + CROWS * (h + 1)] = WCH1

    # q3 fold: z23-half rows -> o3, one 10-col block per chunk slot k.
    # Moving rows for even chunks: [z2(32); z3p(10)]; odd: [z3p(10); z2(32)].
    WH23 = np.zeros([32, 10], np.float64)
    _scatter(WH23, inputs["src3"], inputs["w3"], H2_LO, H2_HI, 0)
    WQ = {0: np.zeros([42, 10], np.float64), 1: np.zeros([42, 10], np.float64)}
    WQ[0][0:32] = WH23
    WQ[0][32:42] = np.eye(10)
    WQ[1][0:10] = np.eye(10)
    WQ[1][10:42] = WH23
    # Stationary base partition must match the moving operand's (64 for
    # odd chunks, whose z23 half lives at rows 64..105).
    for k in range(8):
        base = C_WQ + 80 * k + 10 * k
        rb = 64 * (k % 2)
        wb[rb:rb + 42, base:base + 10] = WQ[k % 2]

    return np.ascontiguousarray(
        _round_dt(wb).astype(NP_DT))


def build_nc() -> bass.Bass:
    nc = bass.Bass()
    wd = nc.declare_dram_parameter("wblob", [128, WCOLS], DT, isOutput=False)
    xd = nc.declare_dram_parameter("xin", [65, BC], DT, isOutput=False)
    o3d = [nc.declare_dram_parameter(f"o3_{g}", [80, CHUNK], F32,
                                     isOutput=True) for g in range(2)]
    if DEBUG:
        dbg_ab = nc.declare_dram_parameter("dbg_ab", [128, 4 * CHUNK], DT,
                                           isOutput=True)
        dbg_c = nc.declare_dram_parameter("dbg_c", [CROWS, CHUNK], DT,
                                          isOutput=True)

    SIG = mybir.ActivationFunctionType.Sigmoid
    NX = BC // 4                # x DMA slice width (2048)

    def cs(c):
        return slice(c * CHUNK, (c + 1) * CHUNK)

    with TileContext(nc) as tc:
        with (
            tc.tile_pool(name="persist", bufs=1) as pp,
            tc.tile_pool(name="pa", bufs=3, space="PSUM") as pa,
            tc.tile_pool(name="pb", bufs=3, space="PSUM") as pb,
            tc.tile_pool(name="pc", bufs=1, space="PSUM") as pc,
            tc.tile_pool(name="pq", bufs=1, space="PSUM") as pq,
        ):
            wb = pp.tile([128, WCOLS], DT)
            xt = pp.tile([65, BC], DT)
            # Parity-split staging: readers of chunk c's sigma only pick up
            # a (whole-tile) dep on the true producing ACT, not this step's.
            st0 = [pp.tile([128, 8 * CHUNK], DT, name=f"st0_{i}")
                   for i in range(2)]   # o0
            st1 = [pp.tile([96, 8 * CHUNK], DT, name=f"st1_{i}")
                   for i in range(2)]    # o1
            st_c = pp.tile([CROWS, 8 * CHUNK], DT)
            o3sb = pp.tile([80, 2 * CHUNK], F32)

            nc.scalar.dma_start(out=wb[:], in_=wd[:])
            for s in range(4):
                nc.sync.dma_start(out=xt[:, s * NX:(s + 1) * NX],
                                  in_=xd[:, s * NX:(s + 1) * NX])

            def o0ap(c, rows=128):
                return st0[c % 2][0:rows, (c // 2) * CHUNK:(c // 2 + 1) * CHUNK]

            def o1ap(c, rows=96):
                return st1[c % 2][0:rows, (c // 2) * CHUNK:(c // 2 + 1) * CHUNK]

            A, B, C, Q = {}, {}, {}, {}
            # Pipeline: z0(c)@c -> ACT-A@c+1 -> z1(c)@c+2 -> ACT-B@c+3 ->
            # z23(c)@c+4 -> pairACT@2p+5 -> folds@2p+6 -> groupACT@8g+12.
            # Every consumer runs >= 1 step after its producer's ACT, so the
            # PE never stalls behind the current step's ACT.
            for t in range(N_CHUNKS + 5):
                # sigmoids first (they unblock next steps, not this one)
                if 1 <= t <= N_CHUNKS:
                    c = t - 1
                    nc.scalar.activation(o0ap(c), A[c][:], SIG, scale=GAIN)
                if 3 <= t <= N_CHUNKS + 2:
                    c = t - 3
                    nc.scalar.activation(o1ap(c), B[c][0:96, :], SIG,
                                         scale=GAIN)

                # z0(t)
                if t <= N_CHUNKS - 1:
                    A[t] = pa.tile([128, CHUNK], F32, name="a")
                    nc.tensor.matmul(A[t][:], wb[0:65, C_W0:C_W0 + 128],
                                     xt[:, cs(t)], start=True, stop=True)
                # z1(t-2)
                if 2 <= t <= N_CHUNKS + 1:
                    c = t - 2
                    bt = B[c] = pb.tile([128, CHUNK], F32, name="b")
                    # o0-pass first (start=True): its RAW dep (this ACT-A)
                    # and the bank WAR (an older ACT-B) share the Activation
                    # sem lane, so they merge into one wait.
                    nc.tensor.matmul(bt[:], wb[0:128, C_W1H:C_W1H + 128],
                                     o0ap(c), start=True, stop=False)
                    nc.tensor.matmul(bt[:], wb[0:65, C_W1X:C_W1X + 128],
                                     xt[:, cs(c)], start=False, stop=True)

                # q3 folds for the pair sigmoided last step
                if t >= 6 and t % 2 == 0 and (p := (t - 6) // 2) <= 7:
                    for c in (2 * p, 2 * p + 1):
                        g, k = c // 8, c % 8
                        if k == 0:
                            Q[g] = pq.tile([80, CHUNK], F32, name="q")
                        h = c % 2
                        nc.tensor.matmul(
                            Q[g][:],
                            wb[64 * h:64 * h + 42,
                               C_WQ + 80 * k:C_WQ + 80 * (k + 1)],
                            st_c[64 * h:64 * h + 42,
                                 p * CHUNK:(p + 1) * CHUNK],
                            start=(k == 0), stop=(k == 7))
                        if k == 7:
                            nc.scalar.activation(
                                o3sb[:, g * CHUNK:(g + 1) * CHUNK],
                                Q[g][:], SIG, scale=GAIN)
                            nc.sync.dma_start(
                                out=o3d[g][:],
                                in_=o3sb[:, g * CHUNK:(g + 1) * CHUNK])

                # z23(t-4)
                if 4 <= t <= N_CHUNKS + 3:
                    c = t - 4
                    p, h = c // 2, c % 2
                    if h == 0:
                        C[p] = pc.tile([CROWS, CHUNK], F32, name="c23")
                    cp = C[p]
                    nc.tensor.matmul(
                        cp[:],
                        wb[0:65, C_WCX + CROWS * h:C_WCX + CROWS * (h + 1)],
                        xt[:, cs(c)], start=(h == 0), stop=False)
                    nc.tensor.matmul(
                        cp[:],
                        wb[0:128, C_WCH0 + CROWS * h:C_WCH0 + CROWS * (h + 1)],
                        o0ap(c), start=False, stop=False)
                    nc.tensor.matmul(
                        cp[:],
                        wb[0:96, C_WCH1 + CROWS * h:C_WCH1 + CROWS * (h + 1)],
                        o1ap(c), start=False, stop=(h == 1))
                    if h == 1:
                        nc.scalar.activation(
                            st_c[:, p * CHUNK:(p + 1) * CHUNK],
                            cp[:], SIG, scale=GAIN)
                        for rb in (32, 64):
                            nc.vector.tensor_copy(
                                st_c[rb:rb + 10, p * CHUNK:(p + 1) * CHUNK],
                                cp[rb:rb + 10, :])
    _fix_sync_waits(nc)
    return nc


def _fix_sync_waits(nc: bass.Bass) -> None:
    """Walrus codegen has ONE sync-wait slot per MM/ACT/DRAIN instruction.
    The tile scheduler emits two waits in three places; each extra wait is
    provably redundant, so drop it:

    (a) A matmul WAW-waits its own PE lane (same PSUM bank, two buffers
    ago) next to a DMA wait. The PE is an in-order FIFO whose sem
    increments follow completion order, and a matmul's first PSUM write
    lands after the previous matmul's last one - program order already
    enforces the WAW.
    (b) A fold matmul waits both the pair-sigmoid ACT and the z3p DVE
    copies. The copies themselves wait that same ACT (WAW on st_c), so
    the DVE wait subsumes the ACT wait.
    (c) The teardown Drain waits every engine's final sem; prune to the
    last output DMA's lane: it waits the last group ACT (<- last fold <-
    everything upstream) and shares the SP HWDGE FIFO with every other
    DMA, so its completion dominates all lanes.
    """
    ins = list(nc.all_instructions())
    act_wait_of_dve = {}   # nth DVE update -> Activation wait value it held
    ndve = 0
    for i in ins:
        si = getattr(i, "sync_info", None)
        if si is None:
            continue
        if any(u.ant_name.startswith("DVE") for u in si.on_update):
            ndve += 1
            for w in si.on_wait:
                if w.ant_name.startswith("Activation"):
                    act_wait_of_dve[ndve] = w.wait_value
    PE_TYPES = ("InstMatmult", "InstLdweights")
    pe_clock = {}   # lane -> max value already waited by an earlier PE inst
    for i in ins:
        tn = type(i).__name__
        si = i.sync_info
        if tn == "InstMatmult" and si and len(si.on_wait) > 1:
            keep = []
            for w in si.on_wait:
                if w.ant_name.startswith("PE"):
                    continue                      # in-order same-engine WAW
                if pe_clock.get(w.ant_name, -1) >= w.wait_value:
                    continue                      # earlier PE inst waited it
                keep.append(w)
            if len(keep) == 2:
                act = [w for w in keep if w.ant_name.startswith("Activation")]
                dve = [w for w in keep if w.ant_name.startswith("DVE")]
                if act and dve:
                    cover = max((v for n, v in act_wait_of_dve.items()
                                 if n <= dve[0].wait_value), default=-1)
                    if cover >= act[0].wait_value:
                        keep = dve                # DVE wait subsumes the ACT
            # LDWEIGHTS-carried waits crash the device; the schedule must
            # leave every matmul with at most one non-redundant wait.
            assert len(keep) <= 1, [(w.ant_name, w.wait_value) for w in keep]
            si.on_wait = keep
            i.sync_info = si
        elif tn == "InstDrain" and si and len(si.on_wait) > 1:
            last_upd = None
            for j in ins:
                if type(j).__name__ == "InstDMACopy" and j.sync_info and \
                        j.outs and "o3_" in str(j.outs[0]):
                    for u in j.sync_info.on_update:
                        last_upd = u.ant_name
            si.on_wait = [w for w in si.on_wait if w.ant_name == last_upd]
            assert len(si.on_wait) == 1, si.on_wait
            i.sync_info = si
        if tn in PE_TYPES and si:
            for w in si.on_wait:
                pe_clock[w.ant_name] = max(pe_clock.get(w.ant_name, -1),
                                           w.wait_value)


def make_in_maps(inputs: dict) -> list[dict]:
    wblob = _build_wblob(inputs)
    x = np.asarray(inputs["x"], np.float32)
    in_maps = []
    for i in range(N_CORES):
        xt = np.empty((65, BC), np.float32)
        xt[0:64, :] = x[i * BC:(i + 1) * BC, :].T
        xt[64, :] = 1.0
        in_maps.append({"wblob": wblob,
                        "xin": np.ascontiguousarray(_round_dt(xt).astype(NP_DT))})
    return in_maps


def assemble_output(results: list[dict]) -> np.ndarray:
    y = np.empty((BATCH, 10), np.float32)
    for i in range(N_CORES):
        for g in range(2):
            o = np.asarray(results[i][f"o3_{g}"], np.float32)  # [80, 512]
            # row 10k+j, col s  ->  sample (8g+k)*512+s, node j
            o = o.reshape(8, 10, CHUNK)
            for k in range(8):
                lo = i * BC + (8 * g + k) * CHUNK
                y[lo:lo + CHUNK, :] = o[k].T
    return y


def assemble_output_core0(results: list[dict]) -> np.ndarray:
    y = np.empty((BC, 10), np.float32)
    for g in range(2):
        o = np.asarray(results[0][f"o3_{g}"], np.float32).reshape(8, 10, CHUNK)
        for k in range(8):
            lo = (8 * g + k) * CHUNK
            y[lo:lo + CHUNK, :] = o[k].T
    return y


def kernel(**inputs: np.ndarray) -> np.ndarray:
    from concourse.bass_utils import run_bass_kernel_spmd

    nc = build_nc()
    in_maps = make_in_maps(inputs)
    res = run_bass_kernel_spmd(nc, in_maps, list(range(N_CORES)))
    return assemble_output(res.results)
